# revision 11
# baseline (speedup 1.0000x reference)
"""Trainium2 Bass kernel for nn_EulerMisorientation3D (v2).

reference math (per voxel, Bunge ZXZ Euler angles scaled by [2pi, pi, 2pi]):
    tr  = sum_i g_ii * g_hat_ii          (elementwise diag product)
    out = mean( arccos(0.5*(tr-1))^2 )

Closed form (alpha=2pi*x0, beta=pi*x1, gamma=2pi*x2):
    u = cos(2pi*s), s = x0+x2;  v = cos(2pi*t), t = x0-x2;  c = cos(pi*x1)
    P4 = 4(1+z) = 2 + 2*c*ch + A*Ah + B*Bh,  A = u*(1+c), B = v*(1-c)
    r  = sqrt(Q4/P4) = tan(theta/2),  Q4 = 8-P4 = 6-G,  G = P4-2
    theta = 2*atan(r),  r = exp(0.5*(ln Q4 - ln P4));  loss = mean(theta^2)

Half-angle trick kills all range wraps: u = 1-2*sin^2(pi*s) with
sin(pi*s-pi) arg within the sin spline domain even for x_hat noise
(|arg| <= ~1.24pi < 4), v = 1-2*sin^2(pi*t), c = -sin(pi*x1 - pi/2).

Engine split:
    PE    s = x0+x2 per tile via two identity-weight fp32r matmuls
          accumulating in PSUM (weights exactly 1 -> near-exact adds).
    Pool  t = x0-x2 per tile; cc = c*ch per span.
    ACT   6 sins/voxel then lnP, lnQ, exp(d/2), atan per span.  Queue is
          pinned in data-arrival order and grouped by table set:
          sin* (trig_and_small) -> ln* (natural_log) -> exp* -> atan*.
    DVE   bf16 chain with flat contiguous APs (2x/4x modes): q=sg^2,
          uv=1-2q, cp/cm, AB, AABB, S1, G=2cc+S1, clamp, d=lnQ-lnP,
          a^2, reduce.  theta^2 = 4*atan(r)^2; host multiplies by 4.

Sharding: voxel axis split over 8 cores; per-core acc[128, SPANS] partial
sums of atan(r)^2; host sums (f64) * 4 / N.
"""

import math

import numpy as np

import concourse.bacc as bacc
import concourse.tile as tile
from concourse.tile_rust import add_dep_helper
from concourse import mybir
from concourse.bass_utils import run_bass_kernel_spmd

F32 = mybir.dt.float32
F32R = mybir.dt.float32r
BF16 = mybir.dt.bfloat16
AF = mybir.ActivationFunctionType
OP = mybir.AluOpType

N_CORES = 8
NVOX = 128 * 128 * 128          # 2097152 voxels
PER = NVOX // N_CORES           # 262144 voxels per core
P = 128
COLS = PER // P                 # 2048
T = 4                           # tiles
C = COLS // T                   # 512
SPANS = T // 2                  # spans of 2 tiles (1024 cols)
W = 2 * C                       # span width

PI = math.pi
LN_EPS = 5e-5
GMAX = 5.99609375               # bf16-exact clamp: Q4 = 6+eps-G > 0


def build_bass():
    nc = bacc.Bacc("TRN2", target_bir_lowering=False, debug=False,
                   num_devices=N_CORES)
    xs = nc.declare_dram_parameter("xs", [3, PER], F32R, isOutput=False)
    xh = nc.declare_dram_parameter("xh", [3, PER], F32R, isOutput=False)
    wid = nc.declare_dram_parameter("wi", [P, 2 * P], F32R, isOutput=False)
    out = nc.declare_dram_parameter("o", [P, SPANS], F32, isOutput=True)

    xs_v = xs[:].rearrange("c (p q) -> p c q", p=P)   # [128, 3, 2048]
    xh_v = xh[:].rearrange("c (p q) -> p c q", p=P)

    with tile.TileContext(nc) as tc:
        with (
            tc.tile_pool(name="io", bufs=2) as io,
            tc.tile_pool(name="wk", bufs=2) as wk,
            tc.tile_pool(name="bt", bufs=2) as bt,
            tc.tile_pool(name="tl", bufs=1) as tl,
            tc.tile_pool(name="big", bufs=1) as big,
            tc.psum_pool(name="ps", bufs=2) as ps,
        ):
            acc = big.tile([P, SPANS], F32, tag="acc")
            b_mpi = big.tile([P, 1], F32, tag="b_mpi")
            b_mpi2 = big.tile([P, 1], F32, tag="b_mpi2")
            b_p2 = big.tile([P, 1], F32, tag="b_p2")
            b_p6 = big.tile([P, 1], F32, tag="b_p6")
            nc.vector.memset(b_mpi, -PI)
            nc.vector.memset(b_mpi2, -PI / 2)
            nc.vector.memset(b_p2, 2.0 + LN_EPS)
            nc.vector.memset(b_p6, 6.0 + LN_EPS)
            w2 = big.tile([P, 2 * P], F32R, tag="w2")
            nc.scalar.dma_start(out=w2, in_=wid[:])
            wi = w2[:, 0:P]
            wni = w2[:, P:2 * P]

            # ---- tiles (allocated up-front; DMAs in arrival order)
            in4s = [io.tile([P, 2, 2, C], F32R, tag="in4", name=f"in4_{j}")
                    for j in range(T)]
            xbs = [bt.tile([P, 2, W], F32R, tag="xb", name=f"xb_{s}")
                   for s in range(SPANS)]

            def dma_tile(j):
                sl = slice(j * C, (j + 1) * C)
                nc.sync.dma_start(out=in4s[j][:, 0], in_=xs_v[:, 0:3:2, sl])
                nc.sync.dma_start(out=in4s[j][:, 1], in_=xh_v[:, 0:3:2, sl])

            def dma_beta(s):
                sl = slice(s * W, (s + 1) * W)
                nc.sync.dma_start(out=xbs[s][:, 0, :], in_=xs_v[:, 1, sl])
                nc.sync.dma_start(out=xbs[s][:, 1, :], in_=xh_v[:, 1, sl])

            dma_tile(0)
            dma_beta(0)
            dma_tile(1)
            dma_beta(1)
            dma_tile(2)
            dma_tile(3)

            # sg/cpcm per tile; sb/cc/g per span
            sgs = [wk.tile([P, 2, 2, C], BF16, tag="sg", name=f"sg_{j}")
                   for j in range(T)]           # [x|h][sig_s|sig_t]
            sbs = [bt.tile([P, 2, W], BF16, tag="sb", name=f"sb_{s}")
                   for s in range(SPANS)]
            ccs = [bt.tile([P, W], BF16, tag="cc", name=f"cc_{s}")
                   for s in range(SPANS)]

            sin_list = []    # ACT sins per tile, in emit order
            tail_chain = []  # ACT tail, grouped by table set

            # ---- per tile: all adds on PE (s and t) -> psum
            for j in range(T):
                in4 = in4s[j]
                pst = ps.tile([P, 2, C], F32, tag="pst")
                ptt = ps.tile([P, 2, C], F32, tag="ptt")
                for k in range(2):
                    nc.tensor.matmul(pst[:, k, :], wi, in4[:, k, 0, :],
                                     start=True, stop=False)
                    nc.tensor.matmul(pst[:, k, :], wi, in4[:, k, 1, :],
                                     start=False, stop=True)
                    nc.tensor.matmul(ptt[:, k, :], wi, in4[:, k, 0, :],
                                     start=True, stop=False)
                    nc.tensor.matmul(ptt[:, k, :], wni, in4[:, k, 1, :],
                                     start=False, stop=True)

                sg = sgs[j]
                sin_list.append(nc.scalar.activation(
                    sg[:, :, 0, :], pst[:], AF.Sin,
                    bias=b_mpi[:], scale=PI))
                sin_list.append(nc.scalar.activation(
                    sg[:, :, 1, :], ptt[:], AF.Sin, bias=0.0, scale=PI))

            # beta sins per span
            beta_sins = []
            for s in range(SPANS):
                ins = nc.scalar.activation(
                    sbs[s][:], xbs[s][:].bitcast(F32), AF.Sin,
                    bias=b_mpi2[:], scale=PI)
                beta_sins.append(ins)

            # ACT sin order: s0,t0,b0,s1,t1,s2,t2,b1,s3,t3
            order = [sin_list[0], sin_list[1], beta_sins[0],
                     sin_list[2], sin_list[3], sin_list[4], sin_list[5],
                     beta_sins[1], sin_list[6], sin_list[7]]

            # ---- tile-grain DVE product chain (flat APs for 2x/4x)
            fl4 = lambda ap: ap.rearrange("p a b w -> p (a b w)")
            fl3 = lambda ap: ap.rearrange("p a w -> p (a w)")
            gbufs = [tl.tile([P, W], BF16, tag=f"g{s}", name=f"g_{s}")
                     for s in range(SPANS)]

            def tile_products(j):
                sp, h = j // 2, j % 2
                csl = slice(h * C, (h + 1) * C)
                sb, sg = sbs[sp], sgs[j]
                qq = wk.tile([P, 2, 2, C], BF16, tag="qq")
                nc.vector.tensor_mul(fl4(qq[:]), fl4(sg[:]), fl4(sg[:]))
                uv = wk.tile([P, 2, 2, C], BF16, tag="uv")
                nc.vector.tensor_scalar(fl4(uv[:]), fl4(qq[:]),
                                        -2.0, 1.0, OP.mult, OP.add)
                cpcm = wk.tile([P, 2, 2, C], BF16, tag="cpcm")
                for k in range(2):
                    nc.vector.tensor_scalar(cpcm[:, k, 0, :], sb[:, k, csl],
                                            -1.0, 1.0, OP.mult, OP.add)
                    nc.vector.tensor_scalar(cpcm[:, k, 1, :], sb[:, k, csl],
                                            1.0, None, OP.add)
                ab = wk.tile([P, 2, 2, C], BF16, tag="ab")
                nc.vector.tensor_mul(fl4(ab[:]), fl4(uv[:]), fl4(cpcm[:]))
                aabb = wk.tile([P, 2, C], BF16, tag="aabb")
                nc.vector.tensor_mul(fl3(aabb[:]),
                                     fl3(ab[:, 0]), fl3(ab[:, 1]))
                s1 = wk.tile([P, C], BF16, tag="s1")
                nc.vector.tensor_add(s1[:], aabb[:, 0, :], aabb[:, 1, :])
                nc.vector.scalar_tensor_tensor(gbufs[sp][:, csl],
                                               ccs[sp][:, csl], 2.0, s1[:],
                                               OP.mult, OP.add)

            def span_cc(sp):
                nc.vector.tensor_mul(ccs[sp][:], sbs[sp][:, 0, :],
                                     sbs[sp][:, 1, :])

            def span_gclamp(sp):
                nc.vector.tensor_scalar(gbufs[sp][:], gbufs[sp][:], GMAX,
                                        None, OP.min)

            # DVE program order tuned for dependency arrival
            span_cc(0)
            tile_products(0)
            tile_products(1)
            span_gclamp(0)
            span_cc(1)
            tile_products(2)
            tile_products(3)
            span_gclamp(1)

            # ---- tail: lnP,lnQ,exp per span (one nle table set),
            # then atans + squares (trig set); d on DVE
            nle_chain, trig_chain = [], []
            rs = []
            for s in range(SPANS):
                lnp = tl.tile([P, W], BF16, tag=f"lnp{s}")
                nle_chain.append(nc.scalar.activation(
                    lnp[:], gbufs[s][:], AF.Ln, bias=b_p2[:], scale=1.0))
                lnq = tl.tile([P, W], BF16, tag=f"lnq{s}")
                nle_chain.append(nc.scalar.activation(
                    lnq[:], gbufs[s][:], AF.Ln, bias=b_p6[:], scale=-1.0))
                d = tl.tile([P, W], BF16, tag=f"d{s}")
                nc.vector.tensor_sub(d[:], lnq[:], lnp[:])
                r = tl.tile([P, W], BF16, tag=f"r{s}")
                nle_chain.append(nc.scalar.activation(
                    r[:], d[:], AF.Exp, bias=0.0, scale=0.5))
                rs.append(r)
            for s in range(SPANS):
                a = tl.tile([P, W], BF16, tag=f"a{s}")
                trig_chain.append(nc.scalar.activation(
                    a[:], rs[s][:], AF.Arctan))
                sq = tl.tile([P, W], BF16, tag=f"sq{s}")
                trig_chain.append(nc.scalar.activation(
                    sq[:], a[:], AF.Square, bias=0.0, scale=1.0,
                    accum_out=acc[:, s:s + 1]))

            # explicit table loads: trig at head, nle before lns, trig
            # again before atans (explicit loads avoid the implicit-load
            # pipeline drain)
            def _load(set_id):
                return nc.scalar.add_instruction(mybir.InstLoadActFuncSet(
                    name=nc.get_next_instruction_name(), ins=[], outs=[],
                    act_func_set_id=set_id))
            ld_trig0 = _load(9)
            ld_nle = _load(6)
            ld_trig1 = _load(9)

            # pin ACT order: sins (arrival), nle group, trig group
            full = ([ld_trig0] + order + [ld_nle] + nle_chain
                    + [ld_trig1] + trig_chain)
            def _raw(x):
                return x.ins if hasattr(x, "ins") else x
            for ai, bi in zip(full, full[1:]):
                add_dep_helper(_raw(bi), _raw(ai), sync=False,
                               reason="ACT table-set ordering")

            nc.sync.dma_start(out=out[:], in_=acc[:])

    nc.compile()
    return nc


_CACHE = {}


def _get_nc():
    if "nc" not in _CACHE:
        _CACHE["nc"] = build_bass()
    return _CACHE["nc"]


def _run(x, x_hat, **spmd_kwargs):
    x = np.ascontiguousarray(np.asarray(x, dtype=np.float32).reshape(3, NVOX))
    xh = np.ascontiguousarray(
        np.asarray(x_hat, dtype=np.float32).reshape(3, NVOX))
    wi = np.concatenate([np.eye(P, dtype=np.float32),
                         -np.eye(P, dtype=np.float32)], axis=1)

    in_maps = []
    for c in range(N_CORES):
        sl = slice(c * PER, (c + 1) * PER)
        in_maps.append({
            "xs": np.ascontiguousarray(x[:, sl]),
            "xh": np.ascontiguousarray(xh[:, sl]),
            "wi": wi,
        })

    nc = _get_nc()
    res = run_bass_kernel_spmd(
        nc, in_maps, core_ids=list(range(N_CORES)), **spmd_kwargs)
    total = 0.0
    for r in res.results:
        total += r["o"].astype(np.float64).sum()
    return np.float32(4.0 * total / NVOX), res


def kernel(x: np.ndarray, x_hat: np.ndarray) -> np.ndarray:
    val, _ = _run(x, x_hat)
    return val


# revision 15
# speedup vs baseline: 1.1062x; 1.1062x over previous
"""Trainium2 Bass kernel for nn_EulerMisorientation3D (v2).

reference math (per voxel, Bunge ZXZ Euler angles scaled by [2pi, pi, 2pi]):
    tr  = sum_i g_ii * g_hat_ii          (elementwise diag product)
    out = mean( arccos(0.5*(tr-1))^2 )

Closed form (alpha=2pi*x0, beta=pi*x1, gamma=2pi*x2):
    u = cos(2pi*s), s = x0+x2;  v = cos(2pi*t), t = x0-x2;  c = cos(pi*x1)
    P4 = 4(1+z) = 2 + 2*c*ch + A*Ah + B*Bh,  A = u*(1+c), B = v*(1-c)
    r  = sqrt(Q4/P4) = tan(theta/2),  Q4 = 8-P4 = 6-G,  G = P4-2
    theta = 2*atan(r),  r = exp(0.5*(ln Q4 - ln P4));  loss = mean(theta^2)

Half-angle trick kills all range wraps: u = 1-2*sin^2(pi*s) with
sin(pi*s-pi) arg within the sin spline domain even for x_hat noise
(|arg| <= ~1.24pi < 4), v = 1-2*sin^2(pi*t), c = -sin(pi*x1 - pi/2).

Engine split:
    PE    s = x0+x2 per tile via two identity-weight fp32r matmuls
          accumulating in PSUM (weights exactly 1 -> near-exact adds).
    Pool  t = x0-x2 per tile; cc = c*ch per span.
    ACT   6 sins/voxel then lnP, lnQ, exp(d/2), atan per span.  Queue is
          pinned in data-arrival order and grouped by table set:
          sin* (trig_and_small) -> ln* (natural_log) -> exp* -> atan*.
    DVE   bf16 chain with flat contiguous APs (2x/4x modes): q=sg^2,
          uv=1-2q, cp/cm, AB, AABB, S1, G=2cc+S1, clamp, d=lnQ-lnP,
          a^2, reduce.  theta^2 = 4*atan(r)^2; host multiplies by 4.

Sharding: voxel axis split over 8 cores; per-core acc[128, SPANS] partial
sums of atan(r)^2; host sums (f64) * 4 / N.
"""

import math

import numpy as np

import concourse.bacc as bacc
import concourse.tile as tile
from concourse.tile_rust import add_dep_helper
from concourse import mybir
from concourse.bass_utils import run_bass_kernel_spmd

F32 = mybir.dt.float32
F32R = mybir.dt.float32r
BF16 = mybir.dt.bfloat16
AF = mybir.ActivationFunctionType
OP = mybir.AluOpType

N_CORES = 8
NVOX = 128 * 128 * 128          # 2097152 voxels
PER = NVOX // N_CORES           # 262144 voxels per core
P = 128
COLS = PER // P                 # 2048
T = 4                           # tiles
C = COLS // T                   # 512
SPANS = T // 2                  # spans of 2 tiles (1024 cols)
W = 2 * C                       # span width

PI = math.pi
LN_EPS = 5e-5
GMAX = 5.99609375               # bf16-exact clamp: Q4 = 6+eps-G > 0


def build_bass():
    nc = bacc.Bacc("TRN2", target_bir_lowering=False, debug=False,
                   num_devices=N_CORES)
    xs = nc.declare_dram_parameter("xs", [3, PER], F32R, isOutput=False)
    xh = nc.declare_dram_parameter("xh", [3, PER], F32R, isOutput=False)
    wid = nc.declare_dram_parameter("wi", [P, 2 * P], F32R, isOutput=False)
    out = nc.declare_dram_parameter("o", [P, SPANS], F32, isOutput=True)

    xs_v = xs[:].rearrange("c (p q) -> p c q", p=P)   # [128, 3, 2048]
    xh_v = xh[:].rearrange("c (p q) -> p c q", p=P)

    with tile.TileContext(nc) as tc:
        with (
            tc.tile_pool(name="io", bufs=2) as io,
            tc.tile_pool(name="wk", bufs=2) as wk,
            tc.tile_pool(name="bt", bufs=2) as bt,
            tc.tile_pool(name="tl", bufs=1) as tl,
            tc.tile_pool(name="big", bufs=1) as big,
            tc.psum_pool(name="ps", bufs=2) as ps,
        ):
            acc = big.tile([P, SPANS], F32, tag="acc")
            b_mpi = big.tile([P, 1], F32, tag="b_mpi")
            b_mpi2 = big.tile([P, 1], F32, tag="b_mpi2")
            b_p2 = big.tile([P, 1], F32, tag="b_p2")
            b_p6 = big.tile([P, 1], F32, tag="b_p6")
            nc.vector.memset(b_mpi, -PI)
            nc.vector.memset(b_mpi2, -PI / 2)
            nc.vector.memset(b_p2, 2.0 + LN_EPS)
            nc.vector.memset(b_p6, 6.0 + LN_EPS)
            w2 = big.tile([P, 2 * P], F32R, tag="w2")
            nc.scalar.dma_start(out=w2, in_=wid[:])
            wi = w2[:, 0:P]
            wni = w2[:, P:2 * P]

            # ---- tiles (allocated up-front; DMAs in arrival order)
            in4s = [io.tile([P, 2, 2, C], F32R, tag="in4", name=f"in4_{j}")
                    for j in range(T)]
            xbs = [bt.tile([P, 2, W], F32R, tag="xb", name=f"xb_{s}")
                   for s in range(SPANS)]

            def dma_tile(j):
                sl = slice(j * C, (j + 1) * C)
                nc.sync.dma_start(out=in4s[j][:, 0], in_=xs_v[:, 0:3:2, sl])
                nc.sync.dma_start(out=in4s[j][:, 1], in_=xh_v[:, 0:3:2, sl])

            def dma_beta(s):
                sl = slice(s * W, (s + 1) * W)
                nc.sync.dma_start(out=xbs[s][:, 0, :], in_=xs_v[:, 1, sl])
                nc.sync.dma_start(out=xbs[s][:, 1, :], in_=xh_v[:, 1, sl])

            dma_tile(0)
            dma_beta(0)
            dma_tile(1)
            dma_beta(1)
            dma_tile(2)
            dma_tile(3)

            # sg per span (sins write strided column slices); sb/cc/g per span
            sgs = [wk.tile([P, 2, 2, W], BF16, tag="sg", name=f"sg_{s}")
                   for s in range(SPANS)]       # [x|h][sig_s|sig_t][W]
            sbs = [bt.tile([P, 2, W], BF16, tag="sb", name=f"sb_{s}")
                   for s in range(SPANS)]
            ccs = [bt.tile([P, W], BF16, tag="cc", name=f"cc_{s}")
                   for s in range(SPANS)]

            sin_list = []    # ACT sins per tile, in emit order
            tail_chain = []  # ACT tail, grouped by table set

            # ---- per tile: all adds on PE (s and t) -> psum
            for j in range(T):
                in4 = in4s[j]
                pst = ps.tile([P, 2, C], F32, tag="pst")
                ptt = ps.tile([P, 2, C], F32, tag="ptt")
                for k in range(2):
                    nc.tensor.matmul(pst[:, k, :], wi, in4[:, k, 0, :],
                                     start=True, stop=False)
                    nc.tensor.matmul(pst[:, k, :], wi, in4[:, k, 1, :],
                                     start=False, stop=True)
                    nc.tensor.matmul(ptt[:, k, :], wi, in4[:, k, 0, :],
                                     start=True, stop=False)
                    nc.tensor.matmul(ptt[:, k, :], wni, in4[:, k, 1, :],
                                     start=False, stop=True)

                sp, h = j // 2, j % 2
                csl = slice(h * C, (h + 1) * C)
                sg = sgs[sp]
                sin_list.append(nc.scalar.activation(
                    sg[:, :, 0, csl], pst[:], AF.Sin,
                    bias=b_mpi[:], scale=PI))
                sin_list.append(nc.scalar.activation(
                    sg[:, :, 1, csl], ptt[:], AF.Sin, bias=0.0, scale=PI))

            # beta sins per span
            beta_sins = []
            for s in range(SPANS):
                ins = nc.scalar.activation(
                    sbs[s][:], xbs[s][:].bitcast(F32), AF.Sin,
                    bias=b_mpi2[:], scale=PI)
                beta_sins.append(ins)

            # ACT sin order: s0,t0,b0,s1,t1,s2,t2,b1,s3,t3
            order = [sin_list[0], sin_list[1], beta_sins[0],
                     sin_list[2], sin_list[3], sin_list[4], sin_list[5],
                     beta_sins[1], sin_list[6], sin_list[7]]

            # ---- span-grain DVE product chain (flat APs for 2x/4x)
            fl4 = lambda ap: ap.rearrange("p a b w -> p (a b w)")
            fl3 = lambda ap: ap.rearrange("p a w -> p (a w)")
            gbufs = [tl.tile([P, W], BF16, tag=f"g{s}", name=f"g_{s}")
                     for s in range(SPANS)]
            ds = [tl.tile([P, W], BF16, tag=f"d{s}", name=f"d_{s}")
                  for s in range(SPANS)]
            lnps = [tl.tile([P, W], BF16, tag=f"lnp{s}", name=f"lnp_{s}")
                    for s in range(SPANS)]
            lnqs = [tl.tile([P, W], BF16, tag=f"lnq{s}", name=f"lnq_{s}")
                    for s in range(SPANS)]

            def span_products(sp, mid=None):
                sb, sg = sbs[sp], sgs[sp]
                cpcm = wk.tile([P, 2, 2, W], BF16, tag="cpcm")
                for k in range(2):
                    nc.vector.tensor_scalar(cpcm[:, k, 0, :], sb[:, k, :],
                                            -1.0, 1.0, OP.mult, OP.add)
                    nc.vector.tensor_scalar(cpcm[:, k, 1, :], sb[:, k, :],
                                            1.0, None, OP.add)
                nc.vector.tensor_mul(ccs[sp][:], sb[:, 0, :], sb[:, 1, :])
                qq = wk.tile([P, 2, 2, W], BF16, tag="qq")
                nc.vector.tensor_mul(fl4(qq[:]), fl4(sg[:]), fl4(sg[:]))
                uv = wk.tile([P, 2, 2, W], BF16, tag="uv")
                nc.vector.tensor_scalar(fl4(uv[:]), fl4(qq[:]),
                                        -2.0, 1.0, OP.mult, OP.add)
                ab = wk.tile([P, 2, 2, W], BF16, tag="ab")
                nc.vector.tensor_mul(fl4(ab[:]), fl4(uv[:]), fl4(cpcm[:]))
                if mid is not None:
                    mid()
                aabb = wk.tile([P, 2, W], BF16, tag="aabb")
                nc.vector.tensor_mul(fl3(aabb[:]),
                                     fl3(ab[:, 0]), fl3(ab[:, 1]))
                s1 = wk.tile([P, W], BF16, tag="s1")
                nc.vector.tensor_add(s1[:], aabb[:, 0, :], aabb[:, 1, :])
                nc.vector.scalar_tensor_tensor(gbufs[sp][:], ccs[sp][:],
                                               2.0, s1[:], OP.mult, OP.add)
                nc.vector.tensor_scalar(gbufs[sp][:], gbufs[sp][:], GMAX,
                                        None, OP.min)

            span_products(0)
            span_products(1)

            # ---- tail: lnP,lnQ,exp per span (one nle table set),
            # then atans + squares (trig set); d on DVE
            nle_chain, trig_chain = [], []
            rs = []
            for s in range(SPANS):
                nle_chain.append(nc.scalar.activation(
                    lnps[s][:], gbufs[s][:], AF.Ln, bias=b_p2[:], scale=1.0))
                nle_chain.append(nc.scalar.activation(
                    lnqs[s][:], gbufs[s][:], AF.Ln, bias=b_p6[:], scale=-1.0))
                nc.vector.tensor_sub(ds[s][:], lnqs[s][:], lnps[s][:])
                r = tl.tile([P, W], BF16, tag=f"r{s}")
                nle_chain.append(nc.scalar.activation(
                    r[:], ds[s][:], AF.Exp, bias=0.0, scale=0.5))
                rs.append(r)
            for s in range(SPANS):
                a = tl.tile([P, W], BF16, tag=f"a{s}")
                trig_chain.append(nc.scalar.activation(
                    a[:], rs[s][:], AF.Arctan))
                sq = tl.tile([P, W], BF16, tag=f"sq{s}")
                trig_chain.append(nc.scalar.activation(
                    sq[:], a[:], AF.Square, bias=0.0, scale=1.0,
                    accum_out=acc[:, s:s + 1]))

            # explicit table loads: trig at head, nle before lns, trig
            # again before atans (explicit loads avoid the implicit-load
            # pipeline drain)
            def _load(set_id):
                return nc.scalar.add_instruction(mybir.InstLoadActFuncSet(
                    name=nc.get_next_instruction_name(), ins=[], outs=[],
                    act_func_set_id=set_id))
            ld_trig0 = _load(9)
            ld_nle = _load(6)
            ld_trig1 = _load(9)

            # pin ACT order: sins (arrival), nle group, trig group
            full = ([ld_trig0] + order + [ld_nle] + nle_chain
                    + [ld_trig1] + trig_chain)
            def _raw(x):
                return x.ins if hasattr(x, "ins") else x
            for ai, bi in zip(full, full[1:]):
                add_dep_helper(_raw(bi), _raw(ai), sync=False,
                               reason="ACT table-set ordering")

            nc.sync.dma_start(out=out[:], in_=acc[:])

    nc.compile()
    return nc


_CACHE = {}


def _get_nc():
    if "nc" not in _CACHE:
        _CACHE["nc"] = build_bass()
    return _CACHE["nc"]


def _run(x, x_hat, **spmd_kwargs):
    x = np.ascontiguousarray(np.asarray(x, dtype=np.float32).reshape(3, NVOX))
    xh = np.ascontiguousarray(
        np.asarray(x_hat, dtype=np.float32).reshape(3, NVOX))
    wi = np.concatenate([np.eye(P, dtype=np.float32),
                         -np.eye(P, dtype=np.float32)], axis=1)

    in_maps = []
    for c in range(N_CORES):
        sl = slice(c * PER, (c + 1) * PER)
        in_maps.append({
            "xs": np.ascontiguousarray(x[:, sl]),
            "xh": np.ascontiguousarray(xh[:, sl]),
            "wi": wi,
        })

    nc = _get_nc()
    res = run_bass_kernel_spmd(
        nc, in_maps, core_ids=list(range(N_CORES)), **spmd_kwargs)
    total = 0.0
    for r in res.results:
        total += r["o"].astype(np.float64).sum()
    return np.float32(4.0 * total / NVOX), res


def kernel(x: np.ndarray, x_hat: np.ndarray) -> np.ndarray:
    val, _ = _run(x, x_hat)
    return val


# revision 16
# speedup vs baseline: 1.1456x; 1.0356x over previous
"""Trainium2 Bass kernel for nn_EulerMisorientation3D (v2).

reference math (per voxel, Bunge ZXZ Euler angles scaled by [2pi, pi, 2pi]):
    tr  = sum_i g_ii * g_hat_ii          (elementwise diag product)
    out = mean( arccos(0.5*(tr-1))^2 )

Closed form (alpha=2pi*x0, beta=pi*x1, gamma=2pi*x2):
    u = cos(2pi*s), s = x0+x2;  v = cos(2pi*t), t = x0-x2;  c = cos(pi*x1)
    P4 = 4(1+z) = 2 + 2*c*ch + A*Ah + B*Bh,  A = u*(1+c), B = v*(1-c)
    r  = sqrt(Q4/P4) = tan(theta/2),  Q4 = 8-P4 = 6-G,  G = P4-2
    theta = 2*atan(r),  r = exp(0.5*(ln Q4 - ln P4));  loss = mean(theta^2)

Half-angle trick kills all range wraps: u = 1-2*sin^2(pi*s) with
sin(pi*s-pi) arg within the sin spline domain even for x_hat noise
(|arg| <= ~1.24pi < 4), v = 1-2*sin^2(pi*t), c = -sin(pi*x1 - pi/2).

Engine split:
    PE    s = x0+x2 per tile via two identity-weight fp32r matmuls
          accumulating in PSUM (weights exactly 1 -> near-exact adds).
    Pool  t = x0-x2 per tile; cc = c*ch per span.
    ACT   6 sins/voxel then lnP, lnQ, exp(d/2), atan per span.  Queue is
          pinned in data-arrival order and grouped by table set:
          sin* (trig_and_small) -> ln* (natural_log) -> exp* -> atan*.
    DVE   bf16 chain with flat contiguous APs (2x/4x modes): q=sg^2,
          uv=1-2q, cp/cm, AB, AABB, S1, G=2cc+S1, clamp, d=lnQ-lnP,
          a^2, reduce.  theta^2 = 4*atan(r)^2; host multiplies by 4.

Sharding: voxel axis split over 8 cores; per-core acc[128, SPANS] partial
sums of atan(r)^2; host sums (f64) * 4 / N.
"""

import math

import numpy as np

import concourse.bacc as bacc
import concourse.tile as tile
from concourse.tile_rust import add_dep_helper
from concourse import mybir
from concourse.bass_utils import run_bass_kernel_spmd

F32 = mybir.dt.float32
F32R = mybir.dt.float32r
BF16 = mybir.dt.bfloat16
AF = mybir.ActivationFunctionType
OP = mybir.AluOpType

N_CORES = 8
NVOX = 128 * 128 * 128          # 2097152 voxels
PER = NVOX // N_CORES           # 262144 voxels per core
P = 128
COLS = PER // P                 # 2048
T = 4                           # tiles
C = COLS // T                   # 512
SPANS = T // 2                  # spans of 2 tiles (1024 cols)
W = 2 * C                       # span width

PI = math.pi
LN_EPS = 5e-5
GMAX = 5.99609375               # bf16-exact clamp: Q4 = 6+eps-G > 0


def build_bass():
    nc = bacc.Bacc("TRN2", target_bir_lowering=False, debug=False,
                   num_devices=N_CORES)
    xs = nc.declare_dram_parameter("xs", [3, PER], F32R, isOutput=False)
    xh = nc.declare_dram_parameter("xh", [3, PER], F32R, isOutput=False)
    wid = nc.declare_dram_parameter("wi", [P, 2 * P], F32R, isOutput=False)
    out = nc.declare_dram_parameter("o", [P, SPANS], F32, isOutput=True)

    xs_v = xs[:].rearrange("c (p q) -> p c q", p=P)   # [128, 3, 2048]
    xh_v = xh[:].rearrange("c (p q) -> p c q", p=P)

    with tile.TileContext(nc) as tc:
        with (
            tc.tile_pool(name="io", bufs=2) as io,
            tc.tile_pool(name="wk", bufs=2) as wk,
            tc.tile_pool(name="bt", bufs=2) as bt,
            tc.tile_pool(name="tl", bufs=1) as tl,
            tc.tile_pool(name="big", bufs=1) as big,
            tc.psum_pool(name="ps", bufs=2) as ps,
        ):
            acc = big.tile([P, SPANS], F32, tag="acc")
            b_mpi = big.tile([P, 1], F32, tag="b_mpi")
            b_mpi2 = big.tile([P, 1], F32, tag="b_mpi2")
            b_p2 = big.tile([P, 1], F32, tag="b_p2")
            b_p6 = big.tile([P, 1], F32, tag="b_p6")
            nc.vector.memset(b_mpi, -PI)
            nc.vector.memset(b_mpi2, -PI / 2)
            nc.vector.memset(b_p2, 2.0 + LN_EPS)
            nc.vector.memset(b_p6, 6.0 + LN_EPS)
            w2 = big.tile([P, 2 * P], F32R, tag="w2")
            nc.scalar.dma_start(out=w2, in_=wid[:])
            wi = w2[:, 0:P]
            wni = w2[:, P:2 * P]

            # ---- tiles (allocated up-front; DMAs in arrival order)
            in4s = [io.tile([P, 2, 2, C], F32R, tag="in4", name=f"in4_{j}")
                    for j in range(T)]
            xbs = [bt.tile([P, 2, W], F32R, tag="xb", name=f"xb_{s}")
                   for s in range(SPANS)]

            def dma_tile(j):
                sl = slice(j * C, (j + 1) * C)
                nc.sync.dma_start(out=in4s[j][:, 0], in_=xs_v[:, 0:3:2, sl])
                nc.sync.dma_start(out=in4s[j][:, 1], in_=xh_v[:, 0:3:2, sl])

            def dma_beta(s):
                sl = slice(s * W, (s + 1) * W)
                nc.sync.dma_start(out=xbs[s][:, 0, :], in_=xs_v[:, 1, sl])
                nc.sync.dma_start(out=xbs[s][:, 1, :], in_=xh_v[:, 1, sl])

            dma_tile(0)
            dma_beta(0)
            dma_tile(1)
            dma_tile(2)
            dma_beta(1)
            dma_tile(3)

            # sg per span (sins write strided column slices); sb/cc/g per span
            sgs = [wk.tile([P, 2, 2, W], BF16, tag="sg", name=f"sg_{s}")
                   for s in range(SPANS)]       # [x|h][sig_s|sig_t][W]
            sbs = [bt.tile([P, 2, W], BF16, tag="sb", name=f"sb_{s}")
                   for s in range(SPANS)]
            ccs = [bt.tile([P, W], BF16, tag="cc", name=f"cc_{s}")
                   for s in range(SPANS)]

            sin_list = []    # ACT sins per tile, in emit order
            tail_chain = []  # ACT tail, grouped by table set

            # ---- per tile: all adds on PE (s and t) -> psum
            for j in range(T):
                in4 = in4s[j]
                pst = ps.tile([P, 2, C], F32, tag="pst")
                ptt = ps.tile([P, 2, C], F32, tag="ptt")
                for k in range(2):
                    nc.tensor.matmul(pst[:, k, :], wi, in4[:, k, 0, :],
                                     start=True, stop=False)
                    nc.tensor.matmul(pst[:, k, :], wi, in4[:, k, 1, :],
                                     start=False, stop=True)
                    nc.tensor.matmul(ptt[:, k, :], wi, in4[:, k, 0, :],
                                     start=True, stop=False)
                    nc.tensor.matmul(ptt[:, k, :], wni, in4[:, k, 1, :],
                                     start=False, stop=True)

                sp, h = j // 2, j % 2
                csl = slice(h * C, (h + 1) * C)
                sg = sgs[sp]
                sin_list.append(nc.scalar.activation(
                    sg[:, :, 0, csl], pst[:], AF.Sin,
                    bias=b_mpi[:], scale=PI))
                sin_list.append(nc.scalar.activation(
                    sg[:, :, 1, csl], ptt[:], AF.Sin, bias=0.0, scale=PI))

            # beta sins per span
            beta_sins = []
            for s in range(SPANS):
                ins = nc.scalar.activation(
                    sbs[s][:], xbs[s][:].bitcast(F32), AF.Sin,
                    bias=b_mpi2[:], scale=PI)
                beta_sins.append(ins)

            # ACT sin order: s0,t0,b0,s1,t1,s2,t2,b1,s3,t3
            order = [sin_list[0], sin_list[1], beta_sins[0],
                     sin_list[2], sin_list[3], sin_list[4], sin_list[5],
                     beta_sins[1], sin_list[6], sin_list[7]]

            # ---- span-grain DVE product chain (flat APs for 2x/4x)
            fl4 = lambda ap: ap.rearrange("p a b w -> p (a b w)")
            fl3 = lambda ap: ap.rearrange("p a w -> p (a w)")
            gbufs = [tl.tile([P, W], BF16, tag=f"g{s}", name=f"g_{s}")
                     for s in range(SPANS)]
            ds = [tl.tile([P, W], BF16, tag=f"d{s}", name=f"d_{s}")
                  for s in range(SPANS)]
            lnps = [tl.tile([P, W], BF16, tag=f"lnp{s}", name=f"lnp_{s}")
                    for s in range(SPANS)]
            lnqs = [tl.tile([P, W], BF16, tag=f"lnq{s}", name=f"lnq_{s}")
                    for s in range(SPANS)]

            def span_products(sp, mid=None):
                sb, sg = sbs[sp], sgs[sp]
                cpcm = wk.tile([P, 2, 2, W], BF16, tag="cpcm")
                for k in range(2):
                    nc.vector.tensor_scalar(cpcm[:, k, 0, :], sb[:, k, :],
                                            -1.0, 1.0, OP.mult, OP.add)
                    nc.vector.tensor_scalar(cpcm[:, k, 1, :], sb[:, k, :],
                                            1.0, None, OP.add)
                nc.vector.tensor_mul(ccs[sp][:], sb[:, 0, :], sb[:, 1, :])
                qq = wk.tile([P, 2, 2, W], BF16, tag="qq")
                nc.vector.tensor_mul(fl4(qq[:]), fl4(sg[:]), fl4(sg[:]))
                uv = wk.tile([P, 2, 2, W], BF16, tag="uv")
                nc.vector.tensor_scalar(fl4(uv[:]), fl4(qq[:]),
                                        -2.0, 1.0, OP.mult, OP.add)
                ab = wk.tile([P, 2, 2, W], BF16, tag="ab")
                nc.vector.tensor_mul(fl4(ab[:]), fl4(uv[:]), fl4(cpcm[:]))
                if mid is not None:
                    mid()
                aabb = wk.tile([P, 2, W], BF16, tag="aabb")
                nc.vector.tensor_mul(fl3(aabb[:]),
                                     fl3(ab[:, 0]), fl3(ab[:, 1]))
                s1 = wk.tile([P, W], BF16, tag="s1")
                nc.vector.tensor_add(s1[:], aabb[:, 0, :], aabb[:, 1, :])
                nc.vector.scalar_tensor_tensor(gbufs[sp][:], ccs[sp][:],
                                               2.0, s1[:], OP.mult, OP.add)
                nc.vector.tensor_scalar(gbufs[sp][:], gbufs[sp][:], GMAX,
                                        None, OP.min)

            span_products(0)
            span_products(1)

            # ---- tail: lnP,lnQ,exp per span (one nle table set),
            # then atans + squares (trig set); d on DVE
            nle_chain, trig_chain = [], []
            rs = []
            for s in range(SPANS):
                nle_chain.append(nc.scalar.activation(
                    lnps[s][:], gbufs[s][:], AF.Ln, bias=b_p2[:], scale=1.0))
                nle_chain.append(nc.scalar.activation(
                    lnqs[s][:], gbufs[s][:], AF.Ln, bias=b_p6[:], scale=-1.0))
                nc.vector.tensor_sub(ds[s][:], lnqs[s][:], lnps[s][:])
                r = tl.tile([P, W], BF16, tag=f"r{s}")
                nle_chain.append(nc.scalar.activation(
                    r[:], ds[s][:], AF.Exp, bias=0.0, scale=0.5))
                rs.append(r)
            for s in range(SPANS):
                a = tl.tile([P, W], BF16, tag=f"a{s}")
                trig_chain.append(nc.scalar.activation(
                    a[:], rs[s][:], AF.Arctan))
                sq = tl.tile([P, W], BF16, tag=f"sq{s}")
                trig_chain.append(nc.scalar.activation(
                    sq[:], a[:], AF.Square, bias=0.0, scale=1.0,
                    accum_out=acc[:, s:s + 1]))

            # explicit table loads (explicit loads avoid the implicit-load
            # pipeline drain).  Per-span tails each load nle then trig so
            # span0's tail completes while span1 products still run; only
            # span1's short tail is exposed at the end.
            def _load(set_id):
                return nc.scalar.add_instruction(mybir.InstLoadActFuncSet(
                    name=nc.get_next_instruction_name(), ins=[], outs=[],
                    act_func_set_id=set_id))
            ld_trig0 = _load(9)
            full = [ld_trig0] + order
            for s2 in range(SPANS):
                full.append(_load(6))
                full.extend(nle_chain[3 * s2:3 * s2 + 3])
                full.append(_load(9))
                full.extend(trig_chain[2 * s2:2 * s2 + 2])
            def _raw(x):
                return x.ins if hasattr(x, "ins") else x
            for ai, bi in zip(full, full[1:]):
                add_dep_helper(_raw(bi), _raw(ai), sync=False,
                               reason="ACT table-set ordering")

            nc.sync.dma_start(out=out[:], in_=acc[:])

    nc.compile()
    return nc


_CACHE = {}


def _get_nc():
    if "nc" not in _CACHE:
        _CACHE["nc"] = build_bass()
    return _CACHE["nc"]


def _run(x, x_hat, **spmd_kwargs):
    x = np.ascontiguousarray(np.asarray(x, dtype=np.float32).reshape(3, NVOX))
    xh = np.ascontiguousarray(
        np.asarray(x_hat, dtype=np.float32).reshape(3, NVOX))
    wi = np.concatenate([np.eye(P, dtype=np.float32),
                         -np.eye(P, dtype=np.float32)], axis=1)

    in_maps = []
    for c in range(N_CORES):
        sl = slice(c * PER, (c + 1) * PER)
        in_maps.append({
            "xs": np.ascontiguousarray(x[:, sl]),
            "xh": np.ascontiguousarray(xh[:, sl]),
            "wi": wi,
        })

    nc = _get_nc()
    res = run_bass_kernel_spmd(
        nc, in_maps, core_ids=list(range(N_CORES)), **spmd_kwargs)
    total = 0.0
    for r in res.results:
        total += r["o"].astype(np.float64).sum()
    return np.float32(4.0 * total / NVOX), res


def kernel(x: np.ndarray, x_hat: np.ndarray) -> np.ndarray:
    val, _ = _run(x, x_hat)
    return val


# revision 17
# speedup vs baseline: 1.1530x; 1.0064x over previous
"""Trainium2 Bass kernel for nn_EulerMisorientation3D (v2).

reference math (per voxel, Bunge ZXZ Euler angles scaled by [2pi, pi, 2pi]):
    tr  = sum_i g_ii * g_hat_ii          (elementwise diag product)
    out = mean( arccos(0.5*(tr-1))^2 )

Closed form (alpha=2pi*x0, beta=pi*x1, gamma=2pi*x2):
    u = cos(2pi*s), s = x0+x2;  v = cos(2pi*t), t = x0-x2;  c = cos(pi*x1)
    P4 = 4(1+z) = 2 + 2*c*ch + A*Ah + B*Bh,  A = u*(1+c), B = v*(1-c)
    r  = sqrt(Q4/P4) = tan(theta/2),  Q4 = 8-P4 = 6-G,  G = P4-2
    theta = 2*atan(r),  r = exp(0.5*(ln Q4 - ln P4));  loss = mean(theta^2)

Half-angle trick kills all range wraps: u = 1-2*sin^2(pi*s) with
sin(pi*s-pi) arg within the sin spline domain even for x_hat noise
(|arg| <= ~1.24pi < 4), v = 1-2*sin^2(pi*t), c = -sin(pi*x1 - pi/2).

Engine split:
    PE    s = x0+x2 per tile via two identity-weight fp32r matmuls
          accumulating in PSUM (weights exactly 1 -> near-exact adds).
    Pool  t = x0-x2 per tile; cc = c*ch per span.
    ACT   6 sins/voxel then lnP, lnQ, exp(d/2), atan per span.  Queue is
          pinned in data-arrival order and grouped by table set:
          sin* (trig_and_small) -> ln* (natural_log) -> exp* -> atan*.
    DVE   bf16 chain with flat contiguous APs (2x/4x modes): q=sg^2,
          uv=1-2q, cp/cm, AB, AABB, S1, G=2cc+S1, clamp, d=lnQ-lnP,
          a^2, reduce.  theta^2 = 4*atan(r)^2; host multiplies by 4.

Sharding: voxel axis split over 8 cores; per-core acc[128, SPANS] partial
sums of atan(r)^2; host sums (f64) * 4 / N.
"""

import math

import numpy as np

import concourse.bacc as bacc
import concourse.tile as tile
from concourse.tile_rust import add_dep_helper
from concourse import mybir
from concourse.bass_utils import run_bass_kernel_spmd

F32 = mybir.dt.float32
F32R = mybir.dt.float32r
BF16 = mybir.dt.bfloat16
AF = mybir.ActivationFunctionType
OP = mybir.AluOpType

N_CORES = 8
NVOX = 128 * 128 * 128          # 2097152 voxels
PER = NVOX // N_CORES           # 262144 voxels per core
P = 128
COLS = PER // P                 # 2048
T = 4                           # tiles
C = COLS // T                   # 512
SPANS = T // 2                  # spans of 2 tiles (1024 cols)
W = 2 * C                       # span width

PI = math.pi
LN_EPS = 5e-5
GMAX = 5.99609375               # bf16-exact clamp: Q4 = 6+eps-G > 0


def build_bass():
    nc = bacc.Bacc("TRN2", target_bir_lowering=False, debug=False,
                   num_devices=N_CORES)
    xs = nc.declare_dram_parameter("xs", [3, PER], F32R, isOutput=False)
    xh = nc.declare_dram_parameter("xh", [3, PER], F32R, isOutput=False)
    wid = nc.declare_dram_parameter("wi", [P, 2 * P], F32R, isOutput=False)
    out = nc.declare_dram_parameter("o", [P, SPANS], F32, isOutput=True)

    xs_v = xs[:].rearrange("c (p q) -> p c q", p=P)   # [128, 3, 2048]
    xh_v = xh[:].rearrange("c (p q) -> p c q", p=P)

    with tile.TileContext(nc) as tc:
        with (
            tc.tile_pool(name="io", bufs=2) as io,
            tc.tile_pool(name="wk", bufs=2) as wk,
            tc.tile_pool(name="bt", bufs=2) as bt,
            tc.tile_pool(name="tl", bufs=1) as tl,
            tc.tile_pool(name="big", bufs=1) as big,
            tc.psum_pool(name="ps", bufs=2) as ps,
        ):
            acc = big.tile([P, SPANS], F32, tag="acc")
            b_mpi = big.tile([P, 1], F32, tag="b_mpi")
            b_mpi2 = big.tile([P, 1], F32, tag="b_mpi2")
            b_p2 = big.tile([P, 1], F32, tag="b_p2")
            b_p6 = big.tile([P, 1], F32, tag="b_p6")
            nc.vector.memset(b_mpi, -PI)
            nc.vector.memset(b_mpi2, -PI / 2)
            nc.vector.memset(b_p2, 2.0 + LN_EPS)
            nc.vector.memset(b_p6, 6.0 + LN_EPS)
            w2 = big.tile([P, 2 * P], F32R, tag="w2")
            nc.scalar.dma_start(out=w2, in_=wid[:])
            wi = w2[:, 0:P]
            wni = w2[:, P:2 * P]

            # ---- tiles (allocated up-front; DMAs in arrival order)
            in4s = [io.tile([P, 2, 2, C], F32R, tag="in4", name=f"in4_{j}")
                    for j in range(T)]
            xbs = [bt.tile([P, 2, W], F32R, tag="xb", name=f"xb_{s}")
                   for s in range(SPANS)]

            def dma_tile(j):
                sl = slice(j * C, (j + 1) * C)
                nc.sync.dma_start(out=in4s[j][:, 0], in_=xs_v[:, 0:3:2, sl])
                nc.sync.dma_start(out=in4s[j][:, 1], in_=xh_v[:, 0:3:2, sl])

            def dma_beta(s):
                sl = slice(s * W, (s + 1) * W)
                nc.sync.dma_start(out=xbs[s][:, 0, :], in_=xs_v[:, 1, sl])
                nc.sync.dma_start(out=xbs[s][:, 1, :], in_=xh_v[:, 1, sl])

            dma_tile(0)
            dma_beta(0)
            dma_tile(1)
            dma_tile(2)
            dma_beta(1)
            dma_tile(3)

            # sg per span (sins write strided column slices); sb/cc/g per span
            sgs = [wk.tile([P, 2, 2, W], BF16, tag="sg", name=f"sg_{s}")
                   for s in range(SPANS)]       # [x|h][sig_s|sig_t][W]
            sbs = [bt.tile([P, 2, W], BF16, tag="sb", name=f"sb_{s}")
                   for s in range(SPANS)]
            ccs = [bt.tile([P, W], BF16, tag="cc", name=f"cc_{s}")
                   for s in range(SPANS)]

            sin_list = []    # ACT sins per tile, in emit order
            tail_chain = []  # ACT tail, grouped by table set

            # ---- per tile: all adds on PE (s and t) -> psum
            for j in range(T):
                in4 = in4s[j]
                pst = ps.tile([P, 2, C], F32, tag="pst")
                ptt = ps.tile([P, 2, C], F32, tag="ptt")
                for k in range(2):
                    nc.tensor.matmul(pst[:, k, :], wi, in4[:, k, 0, :],
                                     start=True, stop=False)
                    nc.tensor.matmul(pst[:, k, :], wi, in4[:, k, 1, :],
                                     start=False, stop=True)
                for k in range(2):
                    nc.tensor.matmul(ptt[:, k, :], wi, in4[:, k, 0, :],
                                     start=True, stop=False)
                    nc.tensor.matmul(ptt[:, k, :], wni, in4[:, k, 1, :],
                                     start=False, stop=True)

                sp, h = j // 2, j % 2
                csl = slice(h * C, (h + 1) * C)
                sg = sgs[sp]
                sin_list.append(nc.scalar.activation(
                    sg[:, :, 0, csl], pst[:], AF.Sin,
                    bias=b_mpi[:], scale=PI))
                sin_list.append(nc.scalar.activation(
                    sg[:, :, 1, csl], ptt[:], AF.Sin, bias=0.0, scale=PI))

            # beta sins per span
            beta_sins = []
            for s in range(SPANS):
                ins = nc.scalar.activation(
                    sbs[s][:], xbs[s][:].bitcast(F32), AF.Sin,
                    bias=b_mpi2[:], scale=PI)
                beta_sins.append(ins)

            # ACT sin order: s0,t0,b0,s1,t1,s2,t2,b1,s3,t3
            order = [sin_list[0], sin_list[1], beta_sins[0],
                     sin_list[2], sin_list[3], sin_list[4], sin_list[5],
                     beta_sins[1], sin_list[6], sin_list[7]]

            # ---- span-grain DVE product chain (flat APs for 2x/4x)
            fl4 = lambda ap: ap.rearrange("p a b w -> p (a b w)")
            fl3 = lambda ap: ap.rearrange("p a w -> p (a w)")
            gbufs = [tl.tile([P, W], BF16, tag=f"g{s}", name=f"g_{s}")
                     for s in range(SPANS)]
            ds = [tl.tile([P, W], BF16, tag=f"d{s}", name=f"d_{s}")
                  for s in range(SPANS)]
            lnps = [tl.tile([P, W], BF16, tag=f"lnp{s}", name=f"lnp_{s}")
                    for s in range(SPANS)]
            lnqs = [tl.tile([P, W], BF16, tag=f"lnq{s}", name=f"lnq_{s}")
                    for s in range(SPANS)]

            def span_products(sp, mid=None):
                sb, sg = sbs[sp], sgs[sp]
                cpcm = wk.tile([P, 2, 2, W], BF16, tag="cpcm")
                for k in range(2):
                    nc.vector.tensor_scalar(cpcm[:, k, 0, :], sb[:, k, :],
                                            -1.0, 1.0, OP.mult, OP.add)
                    nc.vector.tensor_scalar(cpcm[:, k, 1, :], sb[:, k, :],
                                            1.0, None, OP.add)
                nc.vector.tensor_mul(ccs[sp][:], sb[:, 0, :], sb[:, 1, :])
                qq = wk.tile([P, 2, 2, W], BF16, tag="qq")
                nc.vector.tensor_mul(fl4(qq[:]), fl4(sg[:]), fl4(sg[:]))
                uv = wk.tile([P, 2, 2, W], BF16, tag="uv")
                nc.vector.tensor_scalar(fl4(uv[:]), fl4(qq[:]),
                                        -2.0, 1.0, OP.mult, OP.add)
                ab = wk.tile([P, 2, 2, W], BF16, tag="ab")
                nc.vector.tensor_mul(fl4(ab[:]), fl4(uv[:]), fl4(cpcm[:]))
                if mid is not None:
                    mid()
                aabb = wk.tile([P, 2, W], BF16, tag="aabb")
                nc.vector.tensor_mul(fl3(aabb[:]),
                                     fl3(ab[:, 0]), fl3(ab[:, 1]))
                s1 = wk.tile([P, W], BF16, tag="s1")
                nc.vector.tensor_add(s1[:], aabb[:, 0, :], aabb[:, 1, :])
                nc.vector.scalar_tensor_tensor(gbufs[sp][:], ccs[sp][:],
                                               2.0, s1[:], OP.mult, OP.add)
                nc.vector.tensor_scalar(gbufs[sp][:], gbufs[sp][:], GMAX,
                                        None, OP.min)

            span_products(0)
            span_products(1)

            # ---- tail: lnP,lnQ,exp per span (one nle table set),
            # then atans + squares (trig set); d on DVE
            nle_chain, trig_chain = [], []
            rs = []
            for s in range(SPANS):
                nle_chain.append(nc.scalar.activation(
                    lnps[s][:], gbufs[s][:], AF.Ln, bias=b_p2[:], scale=1.0))
                nle_chain.append(nc.scalar.activation(
                    lnqs[s][:], gbufs[s][:], AF.Ln, bias=b_p6[:], scale=-1.0))
                nc.vector.tensor_sub(ds[s][:], lnqs[s][:], lnps[s][:])
                r = tl.tile([P, W], BF16, tag=f"r{s}")
                nle_chain.append(nc.scalar.activation(
                    r[:], ds[s][:], AF.Exp, bias=0.0, scale=0.5))
                rs.append(r)
            for s in range(SPANS):
                a = tl.tile([P, W], BF16, tag=f"a{s}")
                trig_chain.append(nc.scalar.activation(
                    a[:], rs[s][:], AF.Arctan))
                sq = tl.tile([P, W], BF16, tag=f"sq{s}")
                trig_chain.append(nc.scalar.activation(
                    sq[:], a[:], AF.Square, bias=0.0, scale=1.0,
                    accum_out=acc[:, s:s + 1]))

            # explicit table loads (explicit loads avoid the implicit-load
            # pipeline drain).  Per-span tails each load nle then trig so
            # span0's tail completes while span1 products still run; only
            # span1's short tail is exposed at the end.
            def _load(set_id):
                return nc.scalar.add_instruction(mybir.InstLoadActFuncSet(
                    name=nc.get_next_instruction_name(), ins=[], outs=[],
                    act_func_set_id=set_id))
            ld_trig0 = _load(9)
            full = [ld_trig0] + order
            for s2 in range(SPANS):
                full.append(_load(6))
                full.extend(nle_chain[3 * s2:3 * s2 + 3])
                full.append(_load(9))
                full.extend(trig_chain[2 * s2:2 * s2 + 2])
            def _raw(x):
                return x.ins if hasattr(x, "ins") else x
            for ai, bi in zip(full, full[1:]):
                add_dep_helper(_raw(bi), _raw(ai), sync=False,
                               reason="ACT table-set ordering")

            nc.sync.dma_start(out=out[:], in_=acc[:])

    nc.compile()
    return nc


_CACHE = {}


def _get_nc():
    if "nc" not in _CACHE:
        _CACHE["nc"] = build_bass()
    return _CACHE["nc"]


def _run(x, x_hat, **spmd_kwargs):
    x = np.ascontiguousarray(np.asarray(x, dtype=np.float32).reshape(3, NVOX))
    xh = np.ascontiguousarray(
        np.asarray(x_hat, dtype=np.float32).reshape(3, NVOX))
    wi = np.concatenate([np.eye(P, dtype=np.float32),
                         -np.eye(P, dtype=np.float32)], axis=1)

    in_maps = []
    for c in range(N_CORES):
        sl = slice(c * PER, (c + 1) * PER)
        in_maps.append({
            "xs": np.ascontiguousarray(x[:, sl]),
            "xh": np.ascontiguousarray(xh[:, sl]),
            "wi": wi,
        })

    nc = _get_nc()
    res = run_bass_kernel_spmd(
        nc, in_maps, core_ids=list(range(N_CORES)), **spmd_kwargs)
    total = 0.0
    for r in res.results:
        total += r["o"].astype(np.float64).sum()
    return np.float32(4.0 * total / NVOX), res


def kernel(x: np.ndarray, x_hat: np.ndarray) -> np.ndarray:
    val, _ = _run(x, x_hat)
    return val
